# revision 1
# baseline (speedup 1.0000x reference)
"""CrossAttention (cosine-normalized QK) Trainium2 Bass kernel, 8-core SPMD.

Sharding: batch (2) x query-row blocks (4) -> 8 cores. Each core computes the
full K/V projection for its batch (replicated within a batch group) and a
512-row slice of queries; output rows are disjoint, so the gather is a pure
concatenation (no collectives).

v2: K-projection is interleaved with attention in 512-key blocks so the
PE-heavy projection overlaps the ACT-heavy softmax exp; attention partials
accumulate in SBUF fp32.
"""

import numpy as np
import ml_dtypes
from contextlib import ExitStack

import concourse.bacc as bacc
import concourse.bass as bass
import concourse.mybir as mybir
import concourse.tile as tile
from concourse import bass_utils

F32 = mybir.dt.float32
BF16 = mybir.dt.bfloat16
AF = mybir.ActivationFunctionType

B, NQ, NK = 2, 2048, 2048
QD, KD, E, H = 1024, 768, 1024, 16
D = E // H          # 64
NC = 8              # cores
NQC = NQ * B // NC  # 512 query rows per core
SCALE = D ** -0.5   # 0.125
LN_EPS = 1e-5

IC_Q = QD // 128    # 8  contraction chunks for Q proj
IC_K = KD // 128    # 6  contraction chunks for K/V proj
EC = E // 128       # 8  embed chunks
KC = NK // 128      # 16 key chunks
NT = NQC // 128     # 4  query-row tiles
HP = H // 2         # 8  head pairs
KS = 4              # key super-blocks (512 keys each)


def build():
    nc = bacc.Bacc("TRN2", target_bir_lowering=False, debug=False,
                   enable_asserts=False, num_devices=1)

    qT = nc.dram_tensor("qT", [QD, NQC], BF16, kind="ExternalInput").ap()
    kT = nc.dram_tensor("kT", [KD, NK], BF16, kind="ExternalInput").ap()
    vT = nc.dram_tensor("vT", [KD, NK], BF16, kind="ExternalInput").ap()
    wq = nc.dram_tensor("wq", [QD, E], BF16, kind="ExternalInput").ap()
    wk = nc.dram_tensor("wk", [KD, E], BF16, kind="ExternalInput").ap()
    wv = nc.dram_tensor("wv", [KD, E], BF16, kind="ExternalInput").ap()
    wo = nc.dram_tensor("wo", [E, E], BF16, kind="ExternalInput").ap()
    bq = nc.dram_tensor("bq", [E], F32, kind="ExternalInput").ap()
    bk_pp = nc.dram_tensor("bk_pp", [128, EC], F32, kind="ExternalInput").ap()
    bv = nc.dram_tensor("bv", [E], F32, kind="ExternalInput").ap()
    bo = nc.dram_tensor("bo", [E], F32, kind="ExternalInput").ap()
    gam = nc.dram_tensor("gam", [E], F32, kind="ExternalInput").ap()
    bet = nc.dram_tensor("bet", [E], F32, kind="ExternalInput").ap()
    out = nc.dram_tensor("out", [NQC, E], F32, kind="ExternalOutput").ap()

    def bcast_row(vec_ap, parts=128):
        return bass.AP(tensor=vec_ap.tensor, offset=vec_ap.offset,
                       ap=[[0, parts], [1, vec_ap.shape[0]]])

    with tile.TileContext(nc) as tc, ExitStack() as ctx:
        # ---- persistent pools -------------------------------------------
        per = ctx.enter_context(tc.tile_pool(name="per", bufs=1))
        dram = ctx.enter_context(tc.tile_pool(name="dram", bufs=1, space="DRAM"))

        v_sb = per.tile([128, KC, H, D + 1], BF16)      # V with ones col
        kpT_sb = per.tile([128, EC, NK], BF16)          # K proj, transposed
        qnT_sb = per.tile([128, EC, NQC], BF16)         # normalized Q, transposed
        aoT_sb = per.tile([128, EC, NQC], BF16)         # attn out, transposed
        rk_pp = per.tile([128, KC], F32)                # 0.125/||k|| per key
        rk_bf = per.tile([128, KC], BF16)
        ones128 = per.tile([128, 1], BF16)
        nc.vector.memset(ones128, 1.0)
        nc.vector.memset(v_sb[:, :, :, D:D + 1], 1.0)
        eps24 = per.tile([128, 1], F32)
        nc.vector.memset(eps24, 1e-24)
        epsln = per.tile([128, 1], F32)
        nc.vector.memset(epsln, LN_EPS)
        bk_sb = per.tile([128, EC], F32)
        nc.sync.dma_start(out=bk_sb, in_=bk_pp)

        qn_dram = dram.tile([NQC, E], BF16)
        qp_dram = dram.tile([NQC, E], F32)
        rk_dram = dram.tile([1, NK], BF16)

        # ---- phase A: V = value @ Wv + bv  (natural, +ones col) ---------
        with tc.tile_pool(name="pa", bufs=1) as pa, \
             tc.tile_pool(name="psv", bufs=4, space="PSUM") as psv:
            vT_sb = pa.tile([128, IC_K, NK], BF16)
            wv_sb = pa.tile([128, IC_K, E], BF16)
            bv_bc = pa.tile([128, E], F32)
            nc.sync.dma_start(out=vT_sb, in_=vT.rearrange("(c p) n -> p c n", p=128))
            nc.sync.dma_start(out=wv_sb, in_=wv.rearrange("(c p) e -> p c e", p=128))
            nc.gpsimd.dma_start(out=bv_bc, in_=bcast_row(bv))
            for kc in range(KC):
                for ec in range(2):
                    ps_v = psv.tile([128, 512], F32)
                    for ic in range(IC_K):
                        nc.tensor.matmul(ps_v,
                                         vT_sb[:, ic, kc * 128:(kc + 1) * 128],
                                         wv_sb[:, ic, ec * 512:(ec + 1) * 512],
                                         start=(ic == 0), stop=(ic == IC_K - 1))
                    nc.vector.tensor_add(
                        out=v_sb[:, kc, ec * 8:(ec + 1) * 8, 0:D],
                        in0=ps_v.rearrange("p (h d) -> p h d", d=D),
                        in1=bv_bc[:, ec * 512:(ec + 1) * 512].rearrange(
                            "p (h d) -> p h d", d=D))

        # ---- phase C: Qp natural + residual(->DRAM) + Qn^T --------------
        with tc.tile_pool(name="pc", bufs=1) as pc, \
             tc.tile_pool(name="psq", bufs=2, space="PSUM") as psq, \
             tc.tile_pool(name="qsc", bufs=2) as qsc:
            qT_sb = pc.tile([128, IC_Q, NQC], BF16)
            wq_sb = pc.tile([128, IC_Q, E], BF16)
            bq_bc = pc.tile([128, E], F32)
            nc.sync.dma_start(out=qT_sb, in_=qT.rearrange("(c p) n -> p c n", p=128))
            nc.sync.dma_start(out=wq_sb, in_=wq.rearrange("(c p) e -> p c e", p=128))
            nc.gpsimd.dma_start(out=bq_bc, in_=bcast_row(bq))
            for nt in range(NT):
                ps_q = psq.tile([128, E], F32)
                for half in range(2):
                    for ic in range(IC_Q):
                        nc.tensor.matmul(ps_q[:, half * 512:(half + 1) * 512],
                                         qT_sb[:, ic, nt * 128:(nt + 1) * 128],
                                         wq_sb[:, ic, half * 512:(half + 1) * 512],
                                         start=(ic == 0), stop=(ic == IC_Q - 1))
                qp_st = qsc.tile([128, E], F32, tag="qpst")
                nc.vector.tensor_add(out=qp_st, in0=ps_q, in1=bq_bc)
                nc.sync.dma_start(out=qp_dram[nt * 128:(nt + 1) * 128, :], in_=qp_st)
                sq_q = qsc.tile([128, E], F32, tag="sqq")
                nc.vector.tensor_mul(out=sq_q, in0=qp_st, in1=qp_st)
                ssq = qsc.tile([128, 1], F32, tag="ssq")
                nc.vector.reduce_sum(out=ssq, in_=sq_q, axis=mybir.AxisListType.X)
                nc.scalar.activation(out=ssq, in_=ssq, func=AF.Sqrt,
                                     bias=eps24, scale=1.0)
                rq_t = qsc.tile([128, 1], F32, tag="rqt")
                nc.vector.reciprocal(out=rq_t, in_=ssq)
                qn_st = qsc.tile([128, E], BF16, tag="qnst")
                nc.scalar.activation(out=qn_st, in_=qp_st,
                                     func=AF.Identity, scale=rq_t, bias=0.0)
                nc.sync.dma_start(out=qn_dram[nt * 128:(nt + 1) * 128, :], in_=qn_st)
            for ec in range(EC):
                nc.sync.dma_start(out=qnT_sb[:, ec, :],
                                  in_=qn_dram[:, ec * 128:(ec + 1) * 128],
                                  transpose=True)

        # ---- interleaved: K-proj block ks  +  attention over block ks ---
        with tc.tile_pool(name="pb", bufs=1) as pb, \
             tc.tile_pool(name="acp", bufs=1) as acp, \
             tc.tile_pool(name="sqp", bufs=3) as sqp, \
             tc.tile_pool(name="esp", bufs=3) as esp, \
             tc.tile_pool(name="psk", bufs=2, space="PSUM") as psk, \
             tc.tile_pool(name="pss", bufs=1, space="PSUM") as pss, \
             tc.tile_pool(name="ps_s", bufs=1, space="PSUM") as ps_sp, \
             tc.tile_pool(name="ps_o", bufs=2, space="PSUM") as ps_op:
            kT_sb = pb.tile([128, IC_K, NK], BF16)
            wk_sb = pb.tile([128, IC_K, E], BF16)
            nc.sync.dma_start(out=kT_sb, in_=kT.rearrange("(c p) n -> p c n", p=128))
            nc.sync.dma_start(out=wk_sb, in_=wk.rearrange("(c p) e -> p c e", p=128))
            acc = acp.tile([128, H, NQC], F32)   # rows 0..63 outT, row 64 rowsum

            for ks in range(KS):
                # -- K proj for keys [ks*512, (ks+1)*512) --
                ps_ss = pss.tile([1, 512], F32)
                for ec in range(EC):
                    ps_k = psk.tile([128, 512], F32)
                    for ic in range(IC_K):
                        nc.tensor.matmul(ps_k,
                                         wk_sb[:, ic, ec * 128:(ec + 1) * 128],
                                         kT_sb[:, ic, ks * 512:(ks + 1) * 512],
                                         start=(ic == 0), stop=(ic == IC_K - 1))
                    kslice = kpT_sb[:, ec, ks * 512:(ks + 1) * 512]
                    nc.vector.tensor_scalar_add(out=kslice, in0=ps_k,
                                                scalar1=bk_sb[:, ec:ec + 1])
                    sq = sqp.tile([128, 512], BF16)
                    nc.vector.tensor_mul(out=sq, in0=kslice, in1=kslice)
                    nc.tensor.matmul(ps_ss, ones128, sq,
                                     start=(ec == 0), stop=(ec == EC - 1))
                srt = sqp.tile([1, 512], F32, tag="srt")
                nc.scalar.activation(out=srt, in_=ps_ss, func=AF.Sqrt,
                                     bias=eps24[0:1, :], scale=1.0)
                rec = sqp.tile([1, 512], F32, tag="rec")
                nc.vector.reciprocal(out=rec, in_=srt)
                rkb = sqp.tile([1, 512], BF16, tag="rkb")
                nc.scalar.mul(out=rkb, in_=rec, mul=SCALE)
                nc.sync.dma_start(out=rk_dram[:, ks * 512:(ks + 1) * 512], in_=rkb)
                nc.sync.dma_start(
                    out=rk_bf[:, ks * 4:(ks + 1) * 4],
                    in_=rk_dram[:, ks * 512:(ks + 1) * 512].rearrange(
                        "one (a b) -> b (one a)", b=128))
                nc.vector.tensor_copy(out=rk_pp[:, ks * 4:(ks + 1) * 4],
                                      in_=rk_bf[:, ks * 4:(ks + 1) * 4])

                # -- attention over this key block, all head pairs --
                for hp in range(HP):
                    po = [ps_op.tile([D + 1, NQC], F32, tag="po",
                                     name=f"po{ks}_{hp}_{j}") for j in range(2)]
                    for j in range(4):
                        kc = ks * 4 + j
                        ps_s = ps_sp.tile([128, 2 * NQC], F32)
                        for i in range(2):
                            nc.tensor.matmul(
                                ps_s[:, i * NQC:(i + 1) * NQC],
                                kpT_sb[i * D:(i + 1) * D, hp,
                                       kc * 128:(kc + 1) * 128],
                                qnT_sb[i * D:(i + 1) * D, hp, :],
                                start=True, stop=True)
                        es = esp.tile([128, 2 * NQC], BF16)
                        nc.scalar.activation(out=es, in_=ps_s, func=AF.Exp,
                                             scale=rk_pp[:, kc:kc + 1], bias=0.0)
                        for i in range(2):
                            nc.tensor.matmul(po[i],
                                             v_sb[:, kc, 2 * hp + i, :],
                                             es[:, i * NQC:(i + 1) * NQC],
                                             start=(j == 0), stop=(j == 3))
                    for i in range(2):
                        h = 2 * hp + i
                        if ks == 0:
                            nc.vector.tensor_copy(out=acc[0:D + 1, h, :],
                                                  in_=po[i])
                        else:
                            nc.vector.tensor_add(out=acc[0:D + 1, h, :],
                                                 in0=acc[0:D + 1, h, :],
                                                 in1=po[i])

            # -- normalize: aoT = acc / rowsum ----------------------------
            with tc.tile_pool(name="nrm", bufs=4) as nrm, \
                 tc.tile_pool(name="drm", bufs=4, space="DRAM") as drm:
                for h in range(H):
                    rec2 = nrm.tile([1, NQC], F32, tag="rec2")
                    nc.vector.reciprocal(out=rec2, in_=acc[D:D + 1, h, :])
                    rdr = drm.tile([1, NQC], F32)
                    nc.sync.dma_start(out=rdr, in_=rec2)
                    rbc = nrm.tile([D, NQC], F32, tag="rbc")
                    nc.sync.dma_start(
                        out=rbc, in_=bass.AP(tensor=rdr.tensor, offset=rdr.offset,
                                             ap=[[0, D], [1, NQC]]))
                    nc.vector.tensor_mul(
                        out=aoT_sb[(h % 2) * D:(h % 2 + 1) * D, h // 2, :],
                        in0=acc[0:D, h, :], in1=rbc)

        # ---- phase E: out proj + residual + layernorm -------------------
        with tc.tile_pool(name="pe", bufs=1) as pe, \
             tc.tile_pool(name="lnp", bufs=2) as lnp, \
             tc.tile_pool(name="psf", bufs=2, space="PSUM") as psf:
            wo_sb = pe.tile([128, EC, E], BF16)
            bo_bc = pe.tile([128, E], F32)
            gam_bc = pe.tile([128, E], F32)
            bet_bc = pe.tile([128, E], F32)
            nc.sync.dma_start(out=wo_sb, in_=wo.rearrange("(c p) e -> p c e", p=128))
            nc.gpsimd.dma_start(out=bo_bc, in_=bcast_row(bo))
            nc.gpsimd.dma_start(out=gam_bc, in_=bcast_row(gam))
            nc.gpsimd.dma_start(out=bet_bc, in_=bcast_row(bet))
            for nt in range(NT):
                ps_f = psf.tile([128, E], F32)
                for half in range(2):
                    for fc in range(EC):
                        nc.tensor.matmul(ps_f[:, half * 512:(half + 1) * 512],
                                         aoT_sb[:, fc, nt * 128:(nt + 1) * 128],
                                         wo_sb[:, fc, half * 512:(half + 1) * 512],
                                         start=(fc == 0), stop=(fc == EC - 1))
                qp_ld = lnp.tile([128, E], F32, tag="qpld")
                nc.sync.dma_start(out=qp_ld,
                                  in_=qp_dram[nt * 128:(nt + 1) * 128, :])
                xs = lnp.tile([128, E], F32, tag="xs")
                nc.vector.tensor_add(out=xs, in0=ps_f, in1=bo_bc)
                nc.vector.tensor_add(out=xs, in0=xs, in1=qp_ld)
                stats = lnp.tile([128, 2, 6], F32, tag="st")
                xs3 = xs.rearrange("p (a b) -> p a b", b=512)
                for sg in range(2):
                    nc.vector.bn_stats(out=stats[:, sg, :], in_=xs3[:, sg, :])
                mv = lnp.tile([128, 2], F32, tag="mv")
                nc.vector.bn_aggr(out=mv, in_=stats)
                rstd = lnp.tile([128, 1], F32, tag="rstd")
                nc.scalar.activation(out=rstd, in_=mv[:, 1:2], func=AF.Sqrt,
                                     bias=epsln, scale=1.0)
                nc.vector.reciprocal(out=rstd, in_=rstd)
                nmr = lnp.tile([128, 1], F32, tag="nmr")
                nc.vector.tensor_mul(out=nmr, in0=mv[:, 0:1], in1=rstd)
                nc.scalar.mul(out=nmr, in_=nmr, mul=-1.0)
                xn = lnp.tile([128, E], F32, tag="xn")
                nc.scalar.activation(out=xn, in_=xs, func=AF.Identity,
                                     scale=rstd, bias=nmr)
                nc.vector.tensor_mul(out=xn, in0=xn, in1=gam_bc)
                ot = lnp.tile([128, E], F32, tag="ot")
                nc.vector.tensor_add(out=ot, in0=xn, in1=bet_bc)
                nc.sync.dma_start(out=out[nt * 128:(nt + 1) * 128, :], in_=ot)

    nc.compile()
    return nc


_NC_CACHE = None
_last_in_maps = None


def _get_nc():
    global _NC_CACHE
    if _NC_CACHE is None:
        _NC_CACHE = build()
    return _NC_CACHE


def kernel(**inputs):
    q = np.asarray(inputs["query"], np.float32)
    k = np.asarray(inputs["key"], np.float32)
    v = np.asarray(inputs["value"], np.float32)
    Wq = np.asarray(inputs["Wq"], np.float32).astype(ml_dtypes.bfloat16)
    Wk = np.asarray(inputs["Wk"], np.float32).astype(ml_dtypes.bfloat16)
    Wv = np.asarray(inputs["Wv"], np.float32).astype(ml_dtypes.bfloat16)
    Wo = np.asarray(inputs["Wo"], np.float32).astype(ml_dtypes.bfloat16)
    bq = np.asarray(inputs["bq"], np.float32)
    bk = np.asarray(inputs["bk"], np.float32)
    bv = np.asarray(inputs["bv"], np.float32)
    bo = np.asarray(inputs["bo"], np.float32)
    gam = np.asarray(inputs["ln_gamma"], np.float32)
    bet = np.asarray(inputs["ln_beta"], np.float32)

    bk_pp = np.ascontiguousarray(bk.reshape(EC, 128).T)
    kTs = [np.ascontiguousarray(k[b].T.astype(ml_dtypes.bfloat16)) for b in range(B)]
    vTs = [np.ascontiguousarray(v[b].T.astype(ml_dtypes.bfloat16)) for b in range(B)]

    in_maps = []
    for c in range(NC):
        b, r0 = c // 4, (c % 4) * NQC
        qTa = np.ascontiguousarray(q[b, r0:r0 + NQC, :].T.astype(ml_dtypes.bfloat16))
        in_maps.append({
            "qT": qTa, "kT": kTs[b], "vT": vTs[b],
            "wq": Wq, "wk": Wk, "wv": Wv, "wo": Wo,
            "bq": bq, "bk_pp": bk_pp, "bv": bv, "bo": bo,
            "gam": gam, "bet": bet,
        })

    global _last_in_maps
    _last_in_maps = in_maps
    nc = _get_nc()
    res = bass_utils.run_bass_kernel_spmd(nc, in_maps, core_ids=list(range(NC)))

    out = np.empty((B, NQ, E), np.float32)
    for c in range(NC):
        b, r0 = c // 4, (c % 4) * NQC
        out[b, r0:r0 + NQC, :] = res.results[c]["out"]
    return out



# revision 6
# speedup vs baseline: 1.2798x; 1.2798x over previous
"""CrossAttention (cosine-normalized QK) Trainium2 Bass kernel, 8-core SPMD.

Sharding: batch (2) x query-row blocks (4) -> 8 cores. Each core computes the
full K/V projection for its batch (replicated within a batch group) and a
512-row slice of queries; output rows are disjoint, so the gather is a pure
concatenation (no collectives).

v3: dense phase order (V proj -> K proj -> Q proj -> attention -> out proj)
with attention accumulating over all 16 key chunks in PSUM, software-pipelined
scores/exp/av so the PE never stalls on the ACT exp, and all reciprocals
batched into 128-partition layouts via DRAM round-trips.
"""

import numpy as np
import ml_dtypes
from contextlib import ExitStack

import concourse.bacc as bacc
import concourse.bass as bass
import concourse.mybir as mybir
import concourse.tile as tile
from concourse import bass_utils

F32 = mybir.dt.float32
BF16 = mybir.dt.bfloat16
AF = mybir.ActivationFunctionType

B, NQ, NK = 2, 2048, 2048
QD, KD, E, H = 1024, 768, 1024, 16
D = E // H          # 64
NC = 8              # cores
NQC = NQ * B // NC  # 512 query rows per core
SCALE = D ** -0.5   # 0.125
LN_EPS = 1e-5

IC_Q = QD // 128    # 8  contraction chunks for Q proj
IC_K = KD // 128    # 6  contraction chunks for K/V proj
EC = E // 128       # 8  embed chunks
KC = NK // 128      # 16 key chunks
NT = NQC // 128     # 4  query-row tiles
HP = H // 2         # 8  head pairs
KS = 4              # key super-blocks (512 keys each)


def build():
    nc = bacc.Bacc("TRN2", target_bir_lowering=False, debug=False,
                   enable_asserts=False, num_devices=1)

    qT = nc.dram_tensor("qT", [QD, NQC], BF16, kind="ExternalInput").ap()
    kT = nc.dram_tensor("kT", [KD, NK], BF16, kind="ExternalInput").ap()
    vT = nc.dram_tensor("vT", [KD, NK], BF16, kind="ExternalInput").ap()
    wq = nc.dram_tensor("wq", [QD, E], BF16, kind="ExternalInput").ap()
    wk = nc.dram_tensor("wk", [KD, E], BF16, kind="ExternalInput").ap()
    wv = nc.dram_tensor("wv", [KD, E], BF16, kind="ExternalInput").ap()
    wo = nc.dram_tensor("wo", [E, E], BF16, kind="ExternalInput").ap()
    bq = nc.dram_tensor("bq", [E], F32, kind="ExternalInput").ap()
    bk_pp = nc.dram_tensor("bk_pp", [128, EC], F32, kind="ExternalInput").ap()
    bv = nc.dram_tensor("bv", [E], F32, kind="ExternalInput").ap()
    bo = nc.dram_tensor("bo", [E], F32, kind="ExternalInput").ap()
    gam = nc.dram_tensor("gam", [E], F32, kind="ExternalInput").ap()
    bet = nc.dram_tensor("bet", [E], F32, kind="ExternalInput").ap()
    out = nc.dram_tensor("out", [NQC, E], F32, kind="ExternalOutput").ap()

    def bcast_row(vec_ap, parts=128):
        return bass.AP(tensor=vec_ap.tensor, offset=vec_ap.offset,
                       ap=[[0, parts], [1, vec_ap.shape[0]]])

    with tile.TileContext(nc) as tc, ExitStack() as ctx:
        # ---- persistent pools -------------------------------------------
        per = ctx.enter_context(tc.tile_pool(name="per", bufs=1))
        dram = ctx.enter_context(tc.tile_pool(name="dram", bufs=1, space="DRAM"))

        v_sb = per.tile([128, KC, H, D + 1], BF16)      # V with ones col
        kpT_sb = per.tile([128, EC, NK], BF16)          # K proj, transposed
        qnT_sb = per.tile([128, EC, NQC], BF16)         # normalized Q, transposed
        aoT_sb = per.tile([128, EC, NQC], BF16)         # attn out, transposed
        rk_pp = per.tile([128, KC], F32)                # 0.125/||k|| per key
        ones128 = per.tile([128, 1], BF16)
        nc.vector.memset(ones128, 1.0)
        nc.vector.memset(v_sb[:, :, :, D:D + 1], 1.0)
        eps24 = per.tile([128, 1], F32)
        nc.vector.memset(eps24, 1e-24)
        epsln = per.tile([128, 1], F32)
        nc.vector.memset(epsln, LN_EPS)
        bk_sb = per.tile([128, EC], F32)
        nc.sync.dma_start(out=bk_sb, in_=bk_pp)

        qn_dram = dram.tile([NQC, E], BF16)
        qp_dram = dram.tile([NQC, E], F32)
        nsq_dram = dram.tile([1, NK], F32)              # ||k||^2 per key
        rs_dram = dram.tile([H, NQC], F32)              # attn rowsums
        rr_dram = dram.tile([H, NQC], F32)              # 1/rowsum

        # kT/wk and qT/wq staged early so their DMA overlaps phase A
        ldk = ctx.enter_context(tc.tile_pool(name="ldk", bufs=1))
        kT_sb = ldk.tile([128, IC_K, NK], BF16)
        wk_sb = ldk.tile([128, IC_K, E], BF16)
        nc.sync.dma_start(out=kT_sb, in_=kT.rearrange("(c p) n -> p c n", p=128))
        nc.sync.dma_start(out=wk_sb, in_=wk.rearrange("(c p) e -> p c e", p=128))
        ldq = ctx.enter_context(tc.tile_pool(name="ldq", bufs=1))
        qT_sb = ldq.tile([128, IC_Q, NQC], BF16)
        wq_sb = ldq.tile([128, IC_Q, E], BF16)
        nc.sync.dma_start(out=qT_sb, in_=qT.rearrange("(c p) n -> p c n", p=128))
        nc.sync.dma_start(out=wq_sb, in_=wq.rearrange("(c p) e -> p c e", p=128))

        # ---- phase A: V = value @ Wv + bv  (natural, +ones col) ---------
        with tc.tile_pool(name="pa", bufs=1) as pa, \
             tc.tile_pool(name="psv", bufs=4, space="PSUM") as psv:
            vT_sb = pa.tile([128, IC_K, NK], BF16)
            wv_sb = pa.tile([128, IC_K, E], BF16)
            bv_bc = pa.tile([128, E], F32)
            nc.sync.dma_start(out=vT_sb, in_=vT.rearrange("(c p) n -> p c n", p=128))
            nc.sync.dma_start(out=wv_sb, in_=wv.rearrange("(c p) e -> p c e", p=128))
            nc.gpsimd.dma_start(out=bv_bc, in_=bcast_row(bv))
            for kc in range(KC):
                for ec in range(2):
                    ps_v = psv.tile([128, 512], F32)
                    for ic in range(IC_K):
                        nc.tensor.matmul(ps_v,
                                         vT_sb[:, ic, kc * 128:(kc + 1) * 128],
                                         wv_sb[:, ic, ec * 512:(ec + 1) * 512],
                                         start=(ic == 0), stop=(ic == IC_K - 1))
                    nc.vector.tensor_add(
                        out=v_sb[:, kc, ec * 8:(ec + 1) * 8, 0:D],
                        in0=ps_v.rearrange("p (h d) -> p h d", d=D),
                        in1=bv_bc[:, ec * 512:(ec + 1) * 512].rearrange(
                            "p (h d) -> p h d", d=D))

        # ---- phase B: K proj (transposed) + key-norm accumulation -------
        with tc.tile_pool(name="pb", bufs=3) as pb, \
             tc.tile_pool(name="psk", bufs=2, space="PSUM") as psk, \
             tc.tile_pool(name="pss", bufs=2, space="PSUM") as pss:
            for ks in range(KS):
                ps_ss = pss.tile([1, 512], F32)
                for ec in range(EC):
                    ps_k = psk.tile([128, 512], F32)
                    for ic in range(IC_K):
                        nc.tensor.matmul(ps_k,
                                         wk_sb[:, ic, ec * 128:(ec + 1) * 128],
                                         kT_sb[:, ic, ks * 512:(ks + 1) * 512],
                                         start=(ic == 0), stop=(ic == IC_K - 1))
                    kslice = kpT_sb[:, ec, ks * 512:(ks + 1) * 512]
                    nc.vector.tensor_scalar_add(out=kslice, in0=ps_k,
                                                scalar1=bk_sb[:, ec:ec + 1])
                    sq = pb.tile([128, 512], BF16, tag="sq")
                    nc.vector.tensor_mul(out=sq, in0=kslice, in1=kslice)
                    nc.tensor.matmul(ps_ss, ones128, sq,
                                     start=(ec == 0), stop=(ec == EC - 1))
                nsq_sb = pb.tile([1, 512], F32, tag="nsq")
                nc.vector.tensor_copy(out=nsq_sb, in_=ps_ss)
                nc.sync.dma_start(out=nsq_dram[:, ks * 512:(ks + 1) * 512],
                                  in_=nsq_sb)
            # batched 0.125/sqrt(nsq) in [128, KC] layout (key c*128+p -> [p,c])
            nsq_pp = pb.tile([128, KC], F32, tag="npp")
            nc.sync.dma_start(out=nsq_pp,
                              in_=nsq_dram.rearrange("one (c p) -> p (one c)",
                                                     p=128))
            nrm = pb.tile([128, KC], F32, tag="nrm")
            nc.scalar.activation(out=nrm, in_=nsq_pp, func=AF.Sqrt,
                                 bias=eps24, scale=1.0)
            nc.vector.reciprocal(out=nrm, in_=nrm)
            nc.scalar.mul(out=rk_pp, in_=nrm, mul=SCALE)

        # ---- phase C: Qp natural + residual(->DRAM) + Qn^T --------------
        with tc.tile_pool(name="psq", bufs=2, space="PSUM") as psq, \
             tc.tile_pool(name="qsc", bufs=2) as qsc:
            bq_bc = qsc.tile([128, E], F32, tag="bqbc")
            nc.gpsimd.dma_start(out=bq_bc, in_=bcast_row(bq))
            for nt in range(NT):
                ps_q = psq.tile([128, E], F32)
                for half in range(2):
                    for ic in range(IC_Q):
                        nc.tensor.matmul(ps_q[:, half * 512:(half + 1) * 512],
                                         qT_sb[:, ic, nt * 128:(nt + 1) * 128],
                                         wq_sb[:, ic, half * 512:(half + 1) * 512],
                                         start=(ic == 0), stop=(ic == IC_Q - 1))
                qp_st = qsc.tile([128, E], F32, tag="qpst")
                nc.vector.tensor_add(out=qp_st, in0=ps_q, in1=bq_bc)
                nc.sync.dma_start(out=qp_dram[nt * 128:(nt + 1) * 128, :], in_=qp_st)
                sq_q = qsc.tile([128, E], F32, tag="sqq")
                nc.vector.tensor_mul(out=sq_q, in0=qp_st, in1=qp_st)
                ssq = qsc.tile([128, 1], F32, tag="ssq")
                nc.vector.reduce_sum(out=ssq, in_=sq_q, axis=mybir.AxisListType.X)
                nc.scalar.activation(out=ssq, in_=ssq, func=AF.Sqrt,
                                     bias=eps24, scale=1.0)
                rq_t = qsc.tile([128, 1], F32, tag="rqt")
                nc.vector.reciprocal(out=rq_t, in_=ssq)
                qn_st = qsc.tile([128, E], BF16, tag="qnst")
                nc.scalar.activation(out=qn_st, in_=qp_st,
                                     func=AF.Identity, scale=rq_t, bias=0.0)
                nc.sync.dma_start(out=qn_dram[nt * 128:(nt + 1) * 128, :], in_=qn_st)
                for ec in range(EC):
                    nc.sync.dma_start(
                        out=qnT_sb[:, ec, nt * 128:(nt + 1) * 128],
                        in_=qn_dram[nt * 128:(nt + 1) * 128,
                                    ec * 128:(ec + 1) * 128],
                        transpose=True)

        # ---- attention: hp-major, PSUM-accumulated over all 16 kc -------
        # wo + LN params staged here so their DMA overlaps attention
        lde = ctx.enter_context(tc.tile_pool(name="lde", bufs=1))
        wo_sb = lde.tile([128, EC, E], BF16)
        bo_bc = lde.tile([128, E], F32)
        gam_bc = lde.tile([128, E], F32)
        bet_bc = lde.tile([128, E], F32)
        nc.sync.dma_start(out=wo_sb, in_=wo.rearrange("(c p) e -> p c e", p=128))
        nc.gpsimd.dma_start(out=bo_bc, in_=bcast_row(bo))
        nc.gpsimd.dma_start(out=gam_bc, in_=bcast_row(gam))
        nc.gpsimd.dma_start(out=bet_bc, in_=bcast_row(bet))

        with tc.tile_pool(name="esp", bufs=3) as esp, \
             tc.tile_pool(name="nrp", bufs=2) as nrp, \
             tc.tile_pool(name="ps_s", bufs=2, space="PSUM") as ps_sp, \
             tc.tile_pool(name="ps_o", bufs=4, space="PSUM") as ps_op:

            def emit_scores(hp, kc):
                ps_s = ps_sp.tile([128, 2 * NQC], F32, tag="s",
                                  name=f"s{hp}_{kc}")
                for i in range(2):
                    nc.tensor.matmul(
                        ps_s[:, i * NQC:(i + 1) * NQC],
                        kpT_sb[i * D:(i + 1) * D, hp, kc * 128:(kc + 1) * 128],
                        qnT_sb[i * D:(i + 1) * D, hp, :],
                        start=True, stop=True)
                return ps_s

            po = {}
            ps_pend = emit_scores(0, 0)
            for hp in range(HP):
                po[hp] = [ps_op.tile([D + 1, NQC], F32, tag="po",
                                     name=f"po{hp}_{j}")
                          for j in range(2)]
                for kc in range(KC):
                    ps_s = ps_pend
                    es = esp.tile([128, 2 * NQC], BF16)
                    nc.scalar.activation(out=es, in_=ps_s, func=AF.Exp,
                                         scale=rk_pp[:, kc:kc + 1], bias=0.0)
                    # emit next scores before av so the in-order PE queue
                    # has independent work while ACT computes this exp
                    if kc + 1 < KC:
                        ps_pend = emit_scores(hp, kc + 1)
                    elif hp + 1 < HP:
                        ps_pend = emit_scores(hp + 1, 0)
                    for i in range(2):
                        nc.tensor.matmul(po[hp][i],
                                         v_sb[:, kc, 2 * hp + i, :],
                                         es[:, i * NQC:(i + 1) * NQC],
                                         start=(kc == 0), stop=(kc == KC - 1))

                # per-hp normalize: batched reciprocal via DRAM round-trip
                for i in range(2):
                    rs_i = nrp.tile([1, NQC], F32, tag=f"rs{i}")
                    nc.vector.tensor_copy(out=rs_i, in_=po[hp][i][D:D + 1, :])
                    nc.sync.dma_start(
                        out=rs_dram[2 * hp + i:2 * hp + i + 1, :], in_=rs_i)
                rs_pp = nrp.tile([128, 2 * NT], F32, tag="rspp")
                nc.sync.dma_start(
                    out=rs_pp,
                    in_=rs_dram[2 * hp:2 * hp + 2, :].rearrange(
                        "h (c p) -> p (h c)", p=128))
                nc.vector.reciprocal(out=rs_pp, in_=rs_pp)
                nc.sync.dma_start(
                    out=rr_dram[2 * hp:2 * hp + 2, :].rearrange(
                        "h (c p) -> p (h c)", p=128),
                    in_=rs_pp)
                for i in range(2):
                    rbc = nrp.tile([D, NQC], F32, tag=f"rbc{i}")
                    rsrc = rr_dram[2 * hp + i:2 * hp + i + 1, :]
                    nc.sync.dma_start(
                        out=rbc, in_=bass.AP(tensor=rsrc.tensor,
                                             offset=rsrc.offset,
                                             ap=[[0, D], [1, NQC]]))
                    nc.vector.tensor_mul(
                        out=aoT_sb[i * D:(i + 1) * D, hp, :],
                        in0=po[hp][i][0:D, :], in1=rbc)

        # ---- phase E: out proj + residual + layernorm -------------------
        with tc.tile_pool(name="lnp", bufs=2) as lnp, \
             tc.tile_pool(name="psf", bufs=2, space="PSUM") as psf:
            for nt in range(NT):
                ps_f = psf.tile([128, E], F32)
                for half in range(2):
                    for fc in range(EC):
                        nc.tensor.matmul(ps_f[:, half * 512:(half + 1) * 512],
                                         aoT_sb[:, fc, nt * 128:(nt + 1) * 128],
                                         wo_sb[:, fc, half * 512:(half + 1) * 512],
                                         start=(fc == 0), stop=(fc == EC - 1))
                qp_ld = lnp.tile([128, E], F32, tag="qpld")
                nc.sync.dma_start(out=qp_ld,
                                  in_=qp_dram[nt * 128:(nt + 1) * 128, :])
                xs = lnp.tile([128, E], F32, tag="xs")
                nc.vector.tensor_add(out=xs, in0=ps_f, in1=bo_bc)
                nc.vector.tensor_add(out=xs, in0=xs, in1=qp_ld)
                stats = lnp.tile([128, 2, 6], F32, tag="st")
                xs3 = xs.rearrange("p (a b) -> p a b", b=512)
                for sg in range(2):
                    nc.vector.bn_stats(out=stats[:, sg, :], in_=xs3[:, sg, :])
                mv = lnp.tile([128, 2], F32, tag="mv")
                nc.vector.bn_aggr(out=mv, in_=stats)
                rstd = lnp.tile([128, 1], F32, tag="rstd")
                nc.scalar.activation(out=rstd, in_=mv[:, 1:2], func=AF.Sqrt,
                                     bias=epsln, scale=1.0)
                nc.vector.reciprocal(out=rstd, in_=rstd)
                nmr = lnp.tile([128, 1], F32, tag="nmr")
                nc.vector.tensor_mul(out=nmr, in0=mv[:, 0:1], in1=rstd)
                nc.scalar.mul(out=nmr, in_=nmr, mul=-1.0)
                xn = lnp.tile([128, E], F32, tag="xn")
                nc.scalar.activation(out=xn, in_=xs, func=AF.Identity,
                                     scale=rstd, bias=nmr)
                nc.vector.tensor_mul(out=xn, in0=xn, in1=gam_bc)
                ot = lnp.tile([128, E], F32, tag="ot")
                nc.vector.tensor_add(out=ot, in0=xn, in1=bet_bc)
                nc.sync.dma_start(out=out[nt * 128:(nt + 1) * 128, :], in_=ot)

    nc.compile()
    return nc


_NC_CACHE = None
_last_in_maps = None


def _get_nc():
    global _NC_CACHE
    if _NC_CACHE is None:
        _NC_CACHE = build()
    return _NC_CACHE


def kernel(**inputs):
    q = np.asarray(inputs["query"], np.float32)
    k = np.asarray(inputs["key"], np.float32)
    v = np.asarray(inputs["value"], np.float32)
    Wq = np.asarray(inputs["Wq"], np.float32).astype(ml_dtypes.bfloat16)
    Wk = np.asarray(inputs["Wk"], np.float32).astype(ml_dtypes.bfloat16)
    Wv = np.asarray(inputs["Wv"], np.float32).astype(ml_dtypes.bfloat16)
    Wo = np.asarray(inputs["Wo"], np.float32).astype(ml_dtypes.bfloat16)
    bq = np.asarray(inputs["bq"], np.float32)
    bk = np.asarray(inputs["bk"], np.float32)
    bv = np.asarray(inputs["bv"], np.float32)
    bo = np.asarray(inputs["bo"], np.float32)
    gam = np.asarray(inputs["ln_gamma"], np.float32)
    bet = np.asarray(inputs["ln_beta"], np.float32)

    bk_pp = np.ascontiguousarray(bk.reshape(EC, 128).T)
    kTs = [np.ascontiguousarray(k[b].T.astype(ml_dtypes.bfloat16)) for b in range(B)]
    vTs = [np.ascontiguousarray(v[b].T.astype(ml_dtypes.bfloat16)) for b in range(B)]

    in_maps = []
    for c in range(NC):
        b, r0 = c // 4, (c % 4) * NQC
        qTa = np.ascontiguousarray(q[b, r0:r0 + NQC, :].T.astype(ml_dtypes.bfloat16))
        in_maps.append({
            "qT": qTa, "kT": kTs[b], "vT": vTs[b],
            "wq": Wq, "wk": Wk, "wv": Wv, "wo": Wo,
            "bq": bq, "bk_pp": bk_pp, "bv": bv, "bo": bo,
            "gam": gam, "bet": bet,
        })

    global _last_in_maps
    _last_in_maps = in_maps
    nc = _get_nc()
    res = bass_utils.run_bass_kernel_spmd(nc, in_maps, core_ids=list(range(NC)))

    out = np.empty((B, NQ, E), np.float32)
    for c in range(NC):
        b, r0 = c // 4, (c % 4) * NQC
        out[b, r0:r0 + NQC, :] = res.results[c]["out"]
    return out


# revision 8
# speedup vs baseline: 1.3418x; 1.0484x over previous
"""CrossAttention (cosine-normalized QK) Trainium2 Bass kernel, 8-core SPMD.

Sharding: batch (2) x query-row blocks (4) -> 8 cores. Each core computes the
full K/V projection for its batch (replicated within a batch group) and a
512-row slice of queries; output rows are disjoint, so the gather is a pure
concatenation (no collectives).

v4: qnT produced by a second transposed Q-projection on the PE (no DRAM
transpose round-trip), K-proj norm matmuls software-pipelined one step behind
the projection stream, attention PSUM freed immediately into an SBUF staging
copy, per-block key-norm reciprocals, bo folded into the residual, and the
residual prefetched before the output projection.
"""

import numpy as np
import ml_dtypes
from contextlib import ExitStack

import concourse.bacc as bacc
import concourse.bass as bass
import concourse.mybir as mybir
import concourse.tile as tile
from concourse import bass_utils

F32 = mybir.dt.float32
BF16 = mybir.dt.bfloat16
AF = mybir.ActivationFunctionType
ALU = mybir.AluOpType

B, NQ, NK = 2, 2048, 2048
QD, KD, E, H = 1024, 768, 1024, 16
D = E // H          # 64
NC = 8              # cores
NQC = NQ * B // NC  # 512 query rows per core
SCALE = D ** -0.5   # 0.125
LN_EPS = 1e-5

IC_Q = QD // 128    # 8  contraction chunks for Q proj
IC_K = KD // 128    # 6  contraction chunks for K/V proj
EC = E // 128       # 8  embed chunks
KC = NK // 128      # 16 key chunks
NT = NQC // 128     # 4  query-row tiles
HP = H // 2         # 8  head pairs
KS = 4              # key super-blocks (512 keys each)


def build():
    nc = bacc.Bacc("TRN2", target_bir_lowering=False, debug=False,
                   enable_asserts=False, num_devices=1)

    qT = nc.dram_tensor("qT", [QD, NQC], BF16, kind="ExternalInput").ap()
    kT = nc.dram_tensor("kT", [KD, NK], BF16, kind="ExternalInput").ap()
    vT = nc.dram_tensor("vT", [KD, NK], BF16, kind="ExternalInput").ap()
    wq = nc.dram_tensor("wq", [QD, E], BF16, kind="ExternalInput").ap()
    wk = nc.dram_tensor("wk", [KD, E], BF16, kind="ExternalInput").ap()
    wv = nc.dram_tensor("wv", [KD, E], BF16, kind="ExternalInput").ap()
    wo = nc.dram_tensor("wo", [E, E], BF16, kind="ExternalInput").ap()
    bq = nc.dram_tensor("bq", [E], F32, kind="ExternalInput").ap()
    bq_pp = nc.dram_tensor("bq_pp", [128, EC], F32, kind="ExternalInput").ap()
    bk_pp = nc.dram_tensor("bk_pp", [128, EC], F32, kind="ExternalInput").ap()
    bv = nc.dram_tensor("bv", [E], F32, kind="ExternalInput").ap()
    bo = nc.dram_tensor("bo", [E], F32, kind="ExternalInput").ap()
    gam = nc.dram_tensor("gam", [E], F32, kind="ExternalInput").ap()
    bet = nc.dram_tensor("bet", [E], F32, kind="ExternalInput").ap()
    out = nc.dram_tensor("out", [NQC, E], F32, kind="ExternalOutput").ap()

    def bcast_row(vec_ap, parts=128):
        return bass.AP(tensor=vec_ap.tensor, offset=vec_ap.offset,
                       ap=[[0, parts], [1, vec_ap.shape[0]]])

    with tile.TileContext(nc) as tc, ExitStack() as ctx:
        # ---- persistent pools -------------------------------------------
        per = ctx.enter_context(tc.tile_pool(name="per", bufs=1))
        dram = ctx.enter_context(tc.tile_pool(name="dram", bufs=1, space="DRAM"))

        v_sb = per.tile([128, KC, H, D + 1], BF16)      # V with ones col
        kpT_sb = per.tile([128, EC, NK], BF16)          # K proj, transposed
        qnT_sb = per.tile([128, EC, NQC], BF16)         # normalized Q, transposed
        aoT_sb = per.tile([128, EC, NQC], BF16)         # attn out, transposed
        rk_pp = per.tile([128, KC], F32)                # 0.125/||k|| per key
        ones128 = per.tile([128, 1], BF16)
        nc.vector.memset(ones128, 1.0)
        nc.vector.memset(v_sb[:, :, :, D:D + 1], 1.0)
        eps24 = per.tile([128, 1], F32)
        nc.vector.memset(eps24, 1e-24)
        epsln = per.tile([128, 1], F32)
        nc.vector.memset(epsln, LN_EPS)
        bk_sb = per.tile([128, EC], F32)
        nc.sync.dma_start(out=bk_sb, in_=bk_pp)
        bq_sb = per.tile([128, EC], F32)
        nc.sync.dma_start(out=bq_sb, in_=bq_pp)

        qp_dram = dram.tile([NQC, E], F32)              # Qp + bo (residual)
        nsq_dram = dram.tile([1, NK], F32)              # ||k||^2 per key
        rq_dram = dram.tile([1, NQC], F32)              # 1/||q|| per query
        rs_dram = dram.tile([H, NQC], F32)              # attn rowsums
        rr_dram = dram.tile([H, NQC], F32)              # 1/rowsum

        # kT/wk and qT/wq staged early so their DMA overlaps phase A
        ldk = ctx.enter_context(tc.tile_pool(name="ldk", bufs=1))
        kT_sb = ldk.tile([128, IC_K, NK], BF16)
        wk_sb = ldk.tile([128, IC_K, E], BF16)
        nc.sync.dma_start(out=kT_sb, in_=kT.rearrange("(c p) n -> p c n", p=128))
        nc.sync.dma_start(out=wk_sb, in_=wk.rearrange("(c p) e -> p c e", p=128))
        ldq = ctx.enter_context(tc.tile_pool(name="ldq", bufs=1))
        qT_sb = ldq.tile([128, IC_Q, NQC], BF16)
        wq_sb = ldq.tile([128, IC_Q, E], BF16)
        nc.sync.dma_start(out=qT_sb, in_=qT.rearrange("(c p) n -> p c n", p=128))
        nc.sync.dma_start(out=wq_sb, in_=wq.rearrange("(c p) e -> p c e", p=128))

        # ---- phase A: V = value @ Wv + bv  (natural, +ones col) ---------
        with tc.tile_pool(name="pa", bufs=1) as pa, \
             tc.tile_pool(name="psv", bufs=4, space="PSUM") as psv:
            vT_sb = pa.tile([128, IC_K, NK], BF16)
            wv_sb = pa.tile([128, IC_K, E], BF16)
            bv_bc = pa.tile([128, E], F32)
            nc.sync.dma_start(out=vT_sb, in_=vT.rearrange("(c p) n -> p c n", p=128))
            nc.sync.dma_start(out=wv_sb, in_=wv.rearrange("(c p) e -> p c e", p=128))
            nc.gpsimd.dma_start(out=bv_bc, in_=bcast_row(bv))
            for kc in range(KC):
                for ec in range(2):
                    ps_v = psv.tile([128, 512], F32)
                    for ic in range(IC_K):
                        nc.tensor.matmul(ps_v,
                                         vT_sb[:, ic, kc * 128:(kc + 1) * 128],
                                         wv_sb[:, ic, ec * 512:(ec + 1) * 512],
                                         start=(ic == 0), stop=(ic == IC_K - 1))
                    nc.vector.tensor_add(
                        out=v_sb[:, kc, ec * 8:(ec + 1) * 8, 0:D],
                        in0=ps_v.rearrange("p (h d) -> p h d", d=D),
                        in1=bv_bc[:, ec * 512:(ec + 1) * 512].rearrange(
                            "p (h d) -> p h d", d=D))

        # ---- phase C: Qp(+bo) natural -> DRAM; qnT via transposed proj --
        with tc.tile_pool(name="qsc", bufs=2) as qsc, \
             tc.tile_pool(name="psq", bufs=2, space="PSUM") as psq, \
             tc.tile_pool(name="psqt", bufs=2, space="PSUM") as psqt:
            bqo_bc = qsc.tile([128, E], F32, tag="bqo")
            bq_bc = qsc.tile([128, E], F32, tag="bqb")
            nc.gpsimd.dma_start(out=bq_bc, in_=bcast_row(bq))
            nc.gpsimd.dma_start(out=bqo_bc, in_=bcast_row(bo))
            nc.vector.tensor_add(out=bqo_bc, in0=bqo_bc, in1=bq_bc)
            for nt in range(NT):
                ps_q = psq.tile([128, E], F32)
                for half in range(2):
                    for ic in range(IC_Q):
                        nc.tensor.matmul(ps_q[:, half * 512:(half + 1) * 512],
                                         qT_sb[:, ic, nt * 128:(nt + 1) * 128],
                                         wq_sb[:, ic, half * 512:(half + 1) * 512],
                                         start=(ic == 0), stop=(ic == IC_Q - 1))
                # residual written with bq AND bo folded in
                qp_st = qsc.tile([128, E], F32, tag="qpst")
                nc.vector.tensor_add(out=qp_st, in0=ps_q, in1=bqo_bc)
                nc.sync.dma_start(out=qp_dram[nt * 128:(nt + 1) * 128, :],
                                  in_=qp_st)
                # ||q||: from Qp WITHOUT bo (subtract the bo part via bq_bc)
                qp_nb = qsc.tile([128, E], F32, tag="qpnb")
                nc.vector.tensor_add(out=qp_nb, in0=ps_q, in1=bq_bc)
                sq_q = qsc.tile([128, E], F32, tag="sqq")
                nc.vector.tensor_mul(out=sq_q, in0=qp_nb, in1=qp_nb)
                ssq = qsc.tile([128, 1], F32, tag="ssq")
                nc.vector.reduce_sum(out=ssq, in_=sq_q, axis=mybir.AxisListType.X)
                nc.scalar.activation(out=ssq, in_=ssq, func=AF.Sqrt,
                                     bias=eps24, scale=1.0)
                rq_t = qsc.tile([128, 1], F32, tag="rqt")
                nc.vector.reciprocal(out=rq_t, in_=ssq)
                nc.sync.dma_start(
                    out=rq_dram[0:1, nt * 128:(nt + 1) * 128].rearrange(
                        "one p -> p one"),
                    in_=rq_t)
            rq_bc = qsc.tile([128, NQC], F32, tag="rqbc")
            nc.gpsimd.dma_start(
                out=rq_bc,
                in_=bass.AP(tensor=rq_dram.tensor, offset=rq_dram.offset,
                            ap=[[0, 128], [1, NQC]]))
            # transposed projection: qnT[e,q] = (Qp^T + bq) * rq
            for ec in range(EC):
                ps_t = psqt.tile([128, NQC], F32)
                for ic in range(IC_Q):
                    nc.tensor.matmul(ps_t,
                                     wq_sb[:, ic, ec * 128:(ec + 1) * 128],
                                     qT_sb[:, ic, :],
                                     start=(ic == 0), stop=(ic == IC_Q - 1))
                nc.vector.scalar_tensor_tensor(
                    out=qnT_sb[:, ec, :], in0=ps_t,
                    scalar=bq_sb[:, ec:ec + 1], in1=rq_bc,
                    op0=ALU.add, op1=ALU.mult)

        # ---- phase B: K proj (transposed) + key norms, PE-pipelined -----
        with tc.tile_pool(name="pb", bufs=3) as pb, \
             tc.tile_pool(name="psk", bufs=2, space="PSUM") as psk, \
             tc.tile_pool(name="pss", bufs=2, space="PSUM") as pss:

            def norm_chain(ks, ps_ss):
                # emit only after ps_ss's stop matmul: 0.125/sqrt(||k||^2)
                nsq_sb = pb.tile([1, 512], F32, tag="nsq")
                nc.vector.tensor_copy(out=nsq_sb, in_=ps_ss)
                nc.gpsimd.dma_start(out=nsq_dram[:, ks * 512:(ks + 1) * 512],
                                    in_=nsq_sb)
                nsq_pp = pb.tile([128, KC // KS], F32, tag="npp")
                nc.gpsimd.dma_start(
                    out=nsq_pp,
                    in_=nsq_dram[:, ks * 512:(ks + 1) * 512].rearrange(
                        "one (c p) -> p (one c)", p=128))
                nrm = pb.tile([128, KC // KS], F32, tag="nrm")
                nc.scalar.activation(out=nrm, in_=nsq_pp, func=AF.Sqrt,
                                     bias=eps24, scale=1.0)
                nc.vector.reciprocal(out=nrm, in_=nrm)
                kpb = KC // KS
                nc.scalar.mul(out=rk_pp[:, ks * kpb:(ks + 1) * kpb], in_=nrm,
                              mul=SCALE)

            pend = None     # (ps_ss, sq, ec, ks) norm-MM lagging one step
            ss_tiles = {}
            for ks in range(KS):
                ss_tiles[ks] = pss.tile([1, 512], F32, tag="ss",
                                        name=f"ss{ks}")
                for ec in range(EC):
                    ps_k = psk.tile([128, 512], F32)
                    for ic in range(IC_K):
                        nc.tensor.matmul(ps_k,
                                         wk_sb[:, ic, ec * 128:(ec + 1) * 128],
                                         kT_sb[:, ic, ks * 512:(ks + 1) * 512],
                                         start=(ic == 0), stop=(ic == IC_K - 1))
                    if pend is not None:
                        psq_, sq_, ec_, ks_ = pend
                        nc.tensor.matmul(psq_, ones128, sq_,
                                         start=(ec_ == 0), stop=(ec_ == EC - 1))
                        if ec_ == EC - 1:
                            norm_chain(ks_, psq_)
                    kslice = kpT_sb[:, ec, ks * 512:(ks + 1) * 512]
                    nc.vector.tensor_scalar_add(out=kslice, in0=ps_k,
                                                scalar1=bk_sb[:, ec:ec + 1])
                    sq = pb.tile([128, 512], BF16, tag="sq")
                    nc.vector.tensor_mul(out=sq, in0=kslice, in1=kslice)
                    pend = (ss_tiles[ks], sq, ec, ks)
            psq_, sq_, ec_, ks_ = pend
            nc.tensor.matmul(psq_, ones128, sq_,
                             start=(ec_ == 0), stop=(ec_ == EC - 1))
            norm_chain(ks_, psq_)

        # ---- attention: hp-major, PSUM-accumulated over all 16 kc -------
        # wo + LN params + residual staged here to overlap attention
        lde = ctx.enter_context(tc.tile_pool(name="lde", bufs=1))
        wo_sb = lde.tile([128, EC, E], BF16)
        gam_bc = lde.tile([128, E], F32)
        bet_bc = lde.tile([128, E], F32)
        qp_ld = lde.tile([128, NT, E], F32)
        nc.sync.dma_start(out=wo_sb, in_=wo.rearrange("(c p) e -> p c e", p=128))
        nc.gpsimd.dma_start(out=gam_bc, in_=bcast_row(gam))
        nc.gpsimd.dma_start(out=bet_bc, in_=bcast_row(bet))
        nc.gpsimd.dma_start(out=qp_ld,
                            in_=qp_dram.rearrange("(t p) e -> p t e", p=128))

        with tc.tile_pool(name="esp", bufs=3) as esp, \
             tc.tile_pool(name="aor", bufs=4) as aor, \
             tc.tile_pool(name="nrp", bufs=2) as nrp, \
             tc.tile_pool(name="ps_s", bufs=2, space="PSUM") as ps_sp, \
             tc.tile_pool(name="ps_o", bufs=4, space="PSUM") as ps_op:

            def emit_scores(hp, kc):
                ps_s = ps_sp.tile([128, 2 * NQC], F32, tag="s",
                                  name=f"s{hp}_{kc}")
                for i in range(2):
                    nc.tensor.matmul(
                        ps_s[:, i * NQC:(i + 1) * NQC],
                        kpT_sb[i * D:(i + 1) * D, hp, kc * 128:(kc + 1) * 128],
                        qnT_sb[i * D:(i + 1) * D, hp, :],
                        start=True, stop=True)
                return ps_s

            po = {}
            ps_pend = emit_scores(0, 0)
            for hp in range(HP):
                po[hp] = [ps_op.tile([D + 1, NQC], F32, tag="po",
                                     name=f"po{hp}_{j}")
                          for j in range(2)]
                for kc in range(KC):
                    ps_s = ps_pend
                    es = esp.tile([128, 2 * NQC], BF16)
                    nc.scalar.activation(out=es, in_=ps_s, func=AF.Exp,
                                         scale=rk_pp[:, kc:kc + 1], bias=0.0)
                    # emit next scores before av so the in-order PE queue
                    # has independent work while ACT computes this exp
                    if kc + 1 < KC:
                        ps_pend = emit_scores(hp, kc + 1)
                    elif hp + 1 < HP:
                        ps_pend = emit_scores(hp + 1, 0)
                    for i in range(2):
                        nc.tensor.matmul(po[hp][i],
                                         v_sb[:, kc, 2 * hp + i, :],
                                         es[:, i * NQC:(i + 1) * NQC],
                                         start=(kc == 0), stop=(kc == KC - 1))

                # free PSUM immediately: copy po -> SBUF staging
                ao_raw = [aor.tile([D + 1, NQC], F32, tag="ao",
                                   name=f"ao{hp}_{j}") for j in range(2)]
                for i in range(2):
                    nc.vector.tensor_copy(out=ao_raw[i], in_=po[hp][i])
                    nc.gpsimd.dma_start(
                        out=rs_dram[2 * hp + i:2 * hp + i + 1, :],
                        in_=ao_raw[i][D:D + 1, :])
                # batched reciprocal of the two rowsums via DRAM round-trip
                rs_pp = nrp.tile([128, 2 * NT], F32, tag="rspp")
                nc.gpsimd.dma_start(
                    out=rs_pp,
                    in_=rs_dram[2 * hp:2 * hp + 2, :].rearrange(
                        "h (c p) -> p (h c)", p=128))
                nc.vector.reciprocal(out=rs_pp, in_=rs_pp)
                nc.gpsimd.dma_start(
                    out=rr_dram[2 * hp:2 * hp + 2, :].rearrange(
                        "h (c p) -> p (h c)", p=128),
                    in_=rs_pp)
                for i in range(2):
                    rbc = nrp.tile([D, NQC], F32, tag=f"rbc{i}")
                    rsrc = rr_dram[2 * hp + i:2 * hp + i + 1, :]
                    nc.gpsimd.dma_start(
                        out=rbc, in_=bass.AP(tensor=rsrc.tensor,
                                             offset=rsrc.offset,
                                             ap=[[0, D], [1, NQC]]))
                    nc.vector.tensor_mul(
                        out=aoT_sb[i * D:(i + 1) * D, hp, :],
                        in0=ao_raw[i][0:D, :], in1=rbc)

        # ---- phase E: out proj + residual(+bo) + layernorm --------------
        with tc.tile_pool(name="lnp", bufs=2) as lnp, \
             tc.tile_pool(name="psf", bufs=2, space="PSUM") as psf:
            for nt in range(NT):
                ps_f = psf.tile([128, E], F32)
                for half in range(2):
                    for fc in range(EC):
                        nc.tensor.matmul(ps_f[:, half * 512:(half + 1) * 512],
                                         aoT_sb[:, fc, nt * 128:(nt + 1) * 128],
                                         wo_sb[:, fc, half * 512:(half + 1) * 512],
                                         start=(fc == 0), stop=(fc == EC - 1))
                xs = lnp.tile([128, E], F32, tag="xs")
                nc.vector.tensor_add(out=xs, in0=ps_f, in1=qp_ld[:, nt, :])
                stats = lnp.tile([128, 2, 6], F32, tag="st")
                xs3 = xs.rearrange("p (a b) -> p a b", b=512)
                for sg in range(2):
                    nc.vector.bn_stats(out=stats[:, sg, :], in_=xs3[:, sg, :])
                mv = lnp.tile([128, 2], F32, tag="mv")
                nc.vector.bn_aggr(out=mv, in_=stats)
                rstd = lnp.tile([128, 1], F32, tag="rstd")
                nc.scalar.activation(out=rstd, in_=mv[:, 1:2], func=AF.Sqrt,
                                     bias=epsln, scale=1.0)
                nc.vector.reciprocal(out=rstd, in_=rstd)
                nmr = lnp.tile([128, 1], F32, tag="nmr")
                nc.vector.tensor_mul(out=nmr, in0=mv[:, 0:1], in1=rstd)
                nc.scalar.mul(out=nmr, in_=nmr, mul=-1.0)
                xn = lnp.tile([128, E], F32, tag="xn")
                nc.scalar.activation(out=xn, in_=xs, func=AF.Identity,
                                     scale=rstd, bias=nmr)
                nc.vector.tensor_mul(out=xn, in0=xn, in1=gam_bc)
                ot = lnp.tile([128, E], F32, tag="ot")
                nc.vector.tensor_add(out=ot, in0=xn, in1=bet_bc)
                nc.sync.dma_start(out=out[nt * 128:(nt + 1) * 128, :], in_=ot)

    nc.compile()
    return nc


_NC_CACHE = None
_last_in_maps = None


def _get_nc():
    global _NC_CACHE
    if _NC_CACHE is None:
        _NC_CACHE = build()
    return _NC_CACHE


def kernel(**inputs):
    q = np.asarray(inputs["query"], np.float32)
    k = np.asarray(inputs["key"], np.float32)
    v = np.asarray(inputs["value"], np.float32)
    Wq = np.asarray(inputs["Wq"], np.float32).astype(ml_dtypes.bfloat16)
    Wk = np.asarray(inputs["Wk"], np.float32).astype(ml_dtypes.bfloat16)
    Wv = np.asarray(inputs["Wv"], np.float32).astype(ml_dtypes.bfloat16)
    Wo = np.asarray(inputs["Wo"], np.float32).astype(ml_dtypes.bfloat16)
    bq = np.asarray(inputs["bq"], np.float32)
    bk = np.asarray(inputs["bk"], np.float32)
    bv = np.asarray(inputs["bv"], np.float32)
    bo = np.asarray(inputs["bo"], np.float32)
    gam = np.asarray(inputs["ln_gamma"], np.float32)
    bet = np.asarray(inputs["ln_beta"], np.float32)

    bq_pp = np.ascontiguousarray(bq.reshape(EC, 128).T)
    bk_pp = np.ascontiguousarray(bk.reshape(EC, 128).T)
    kTs = [np.ascontiguousarray(k[b].T.astype(ml_dtypes.bfloat16)) for b in range(B)]
    vTs = [np.ascontiguousarray(v[b].T.astype(ml_dtypes.bfloat16)) for b in range(B)]

    in_maps = []
    for c in range(NC):
        b, r0 = c // 4, (c % 4) * NQC
        qTa = np.ascontiguousarray(q[b, r0:r0 + NQC, :].T.astype(ml_dtypes.bfloat16))
        in_maps.append({
            "qT": qTa, "kT": kTs[b], "vT": vTs[b],
            "wq": Wq, "wk": Wk, "wv": Wv, "wo": Wo,
            "bq": bq, "bq_pp": bq_pp, "bk_pp": bk_pp, "bv": bv, "bo": bo,
            "gam": gam, "bet": bet,
        })

    global _last_in_maps
    _last_in_maps = in_maps
    nc = _get_nc()
    res = bass_utils.run_bass_kernel_spmd(nc, in_maps, core_ids=list(range(NC)))

    out = np.empty((B, NQ, E), np.float32)
    for c in range(NC):
        b, r0 = c // 4, (c % 4) * NQC
        out[b, r0:r0 + NQC, :] = res.results[c]["out"]
    return out
